# revision 38
# baseline (speedup 1.0000x reference)
"""CRF negative-log-likelihood loss (BERT_BiLSTM_CRF) on 8 TRN2 NeuronCores.

v4: rank-1 segment factorization (data-parallel over batch, 64 seqs/core).

The linear-space forward recursion a_t = D_t M a_{t-1} is split into
NSEG=8 segments of L=256 steps. Products of positive matrices contract
to rank-1 (Birkhoff), so each segment's transfer matrix P_k is summarized
by probe chains computed with the SAME per-tick kernel structure:
  f_k = P_k e   (forward chain; f_1 starts from the true one-hot START)
  h_k: u <- D_t M^T u over the segment, descending   ((M^T e)^T P_k = h_k^T M)
Stitch:  logZ = log(w^T f_m) + sum_{k>=2}[log(h_k^T M f_{k-1})
                                          - log(h_k^T M e)] + offsets + MU*S
(verified exact to 1e-11 in fp64 on the real data: the rank-2 component
of a 256-step product is ~0).

All 15 chains run CONCURRENTLY: state is [64, 512] bf16 = (fwd tags |
bwd tags) x (segment, sequence), so the serial scan is 256 ticks of one
bf16 matmul [64,512] + one DVE multiply — 4x fewer serial round trips
than meet-in-the-middle. Per tick, 8 PE transposes + 1 batched ACT exp
produce the tick's xt [64,512] from chunked emission DMAs (fwd natural,
bwd reversed), a chunk ahead of use. Gold emission score via one-hot
pieces on DVE spread between ticks; renorm once at tick 127 by proxy-row
reciprocal (h-chain scale offsets cancel in the stitch, so only f-chain
logs accumulate).
"""
import numpy as np

TAGSET = 32
START = 30
STOP = 31
B = 512
S = 2048
NCORES = 8
BC = B // NCORES          # 64 sequences per core
NSEG = 8                  # segments (rank-1 factorization)
L = S // NSEG             # 256 serial ticks
CH = 16                   # ticks per emission chunk
NCHK = L // CH            # 16 chunks
MU = np.float32(4.3226)   # mean log-growth per step (measured offline)

_CACHE = {}


def _build_nc(debug=False, gold=2, reps=1):
    import concourse.bacc as bacc
    import concourse.bass as bass
    import concourse.tile as tile
    from concourse import mybir

    f32 = mybir.dt.float32
    bf16 = mybir.dt.bfloat16
    i32 = mybir.dt.int32
    AF = mybir.ActivationFunctionType
    OP = mybir.AluOpType
    AX = mybir.AxisListType

    nc = bacc.Bacc("TRN2", target_bir_lowering=False, debug=False,
                   num_devices=NCORES)

    em_d = nc.dram_tensor("emissions", [BC, S, TAGSET], f32,
                          kind="ExternalInput").ap()
    tg_d = nc.dram_tensor("tags", [BC, S], i32, kind="ExternalInput").ap()
    tr_d = nc.dram_tensor("transitions", [TAGSET, TAGSET], f32,
                          kind="ExternalInput").ap()
    nll_d = nc.dram_tensor("nll", [1, BC], f32, kind="ExternalOutput").ap()

    NC = NSEG * 64            # state columns
    # emissions viewed [b, (seg, step), tag]
    em_r = em_d.rearrange("b (j s) t -> b s j t", j=NSEG)

    with tile.TileContext(nc) as tc:
        with (
            tc.tile_pool(name="const", bufs=1) as cp,
            tc.tile_pool(name="chunk", bufs=2) as ccp,
            tc.tile_pool(name="oh", bufs=2) as ohp,
            tc.tile_pool(name="xt", bufs=CH + 3) as xtp,
            tc.tile_pool(name="state", bufs=3) as stp,
            tc.tile_pool(name="small", bufs=2) as smp,
            tc.tile_pool(name="trp", bufs=3, space="PSUM") as trp,
            tc.tile_pool(name="mmp", bufs=2, space="PSUM") as mmp,
            tc.tile_pool(name="finp", bufs=1, space="PSUM") as fip,
        ):
            _loop = None
            if reps > 1:
                _loop = tc.For_i(
                    0, reps, 1,
                    hint_engines=(mybir.EngineType.PE, mybir.EngineType.DVE,
                                  mybir.EngineType.Activation))
                _loop.__enter__()

            # ---------------- setup: weights, identity, ones ----------------
            w = cp.tile([64, 64], f32)
            nc.vector.memset(w[:], 0.0)
            # fwd block: w[p, t] = trans[t, p] -> applies M = exp(trans)
            nc.sync.dma_start(w[0:32, 0:32], tr_d.rearrange("a b -> b a"))
            # bwd block: w[32+p, 32+t] = trans[p, t] -> applies M^T
            nc.sync.dma_start(w[32:64, 32:64], tr_d)
            nc.vector.tensor_scalar_max(w[:], w[:], -80.0)
            nc.scalar.activation(w[:], w[:], AF.Exp)
            nc.vector.memset(w[0:32, 32:64], 0.0)
            nc.vector.memset(w[32:64, 0:32], 0.0)
            wb16 = cp.tile([64, 64], bf16)
            nc.vector.tensor_copy(wb16[:], w[:])

            ones_t = cp.tile([64, 64], f32)
            nc.vector.memset(ones_t[:], 1.0)
            negmu = cp.tile([64, 1], f32)
            nc.vector.memset(negmu[:], -float(MU))
            ident = cp.tile([64, 64], f32)
            nc.gpsimd.affine_select(
                out=ident[:], in_=ones_t[:], pattern=[[-1, 64]],
                compare_op=OP.is_equal, fill=0.0, base=0, channel_multiplier=1)
            ident_b = cp.tile([64, 64], bf16)
            nc.vector.tensor_copy(ident_b[:], ident[:])

            # stitch constants: c = M e (bwd-block row sums), w_stop vector
            cvec = cp.tile([64, 1], f32)
            nc.vector.tensor_reduce(cvec[32:64, :], w[32:64, 32:64],
                                    axis=AX.X, op=OP.add)
            c0 = cp.tile([32, 1], f32)
            nc.sync.dma_start(c0[:], cvec[32:64, :])
            wv = cp.tile([32, 1], f32)
            nc.sync.dma_start(wv[:], tr_d[STOP:STOP + 1, :]
                              .rearrange("a b -> b a"))
            nc.vector.tensor_scalar_max(wv[:], wv[:], -80.0)
            nc.scalar.activation(wv[:], wv[:], AF.Exp)

            # ---------------- gold one-hot machinery ----------------
            i16 = mybir.dt.int16
            tags_sb = cp.tile([BC, S], i32)
            nc.sync.dma_start(tags_sb[:], tg_d)
            tags16 = cp.tile([BC, S], i16)
            nc.vector.tensor_copy(tags16[:], tags_sb[:])
            iota_t = cp.tile([BC, CH * TAGSET], i32)
            nc.gpsimd.iota(iota_t[:], pattern=[[0, CH], [1, TAGSET]], base=0,
                           channel_multiplier=0)
            iota16 = cp.tile([BC, CH * TAGSET], i16)
            nc.vector.tensor_copy(iota16[:], iota_t[:])
            NACC = NCHK * NSEG
            acc_e = cp.tile([BC, NACC], f32)
            nc.vector.memset(acc_e[:], 0.0)

            # ---------------- state init ----------------
            offacc = cp.tile([64, NC], f32)
            nc.vector.memset(offacc[:], 0.0)
            ones_b = cp.tile([64, 64], bf16)
            nc.vector.memset(ones_b[:], 1.0)

            state = stp.tile([64, NC], bf16, tag="state")
            nc.vector.memset(state[:], 1.0)
            # f_1 init: one-hot START on fwd group 0
            nc.gpsimd.affine_select(
                out=state[0:32, 0:64], in_=ones_b[0:32, :], pattern=[[0, 64]],
                compare_op=OP.is_equal, fill=0.0, base=-START,
                channel_multiplier=1)

            # ---------------- chunk machinery ----------------
            # comb(c): [BC, CH*NSEG*64] f32, dims (l, j, u, t):
            #   u=0: emissions[:, j*L + c*CH + l, :]          (fwd, ascending)
            #   u=1: emissions[:, (j+1)*L - 1 - c*CH - l, :]  (bwd, descending)
            def load_chunk(c):
                comb = ccp.tile([BC, CH * NSEG * 64], f32, tag="comb")
                cv = comb[:].rearrange("b (l j u t) -> b l j u t",
                                       j=NSEG, u=2, t=TAGSET)
                hi = L - 1 - c * CH
                lo = L - (c + 1) * CH - 1
                bsl = slice(hi, None, -1) if lo < 0 else slice(hi, lo, -1)
                for j in range(NSEG):
                    nc.sync.dma_start(cv[:, :, j, 0, :],
                                      em_r[:, c * CH:(c + 1) * CH, j, :])
                    nc.sync.dma_start(cv[:, :, j, 1, :], em_r[:, bsl, j, :])
                # bf16 copy of the fwd halves for the 16-bit gold pipeline
                combx = ccp.tile([BC, CH * NSEG * TAGSET], bf16, tag="combx")
                xv = combx[:].rearrange("b (l j t) -> b l j t",
                                        j=NSEG, t=TAGSET)
                nc.scalar.activation(xv[:, :, 0:4, :],
                                     cv[:, :, 0:4, 0, :], AF.Copy)
                nc.scalar.activation(xv[:, :, 4:8, :],
                                     cv[:, :, 4:8, 0, :], AF.Copy)
                return comb, combx

            # per-tick xt: 8 transposes [64,64] -> one PSUM bank -> one exp
            def prep_tick(comb, l):
                trg = trp.tile([64, NC], f32, tag="trg")
                for j in range(NSEG):
                    nc.tensor.transpose(
                        trg[:, j * 64:(j + 1) * 64],
                        comb[:, (l * NSEG + j) * 64:(l * NSEG + j + 1) * 64],
                        ident[:])
                xt = xtp.tile([64, NC], bf16, tag="xt")
                nc.scalar.activation(xt[:], trg[:], AF.Exp, bias=negmu[:])
                return xt

            def gold_piece(combx, c, j):
                # one-hot gold for chunk c, segment j (fwd steps only; the
                # fwd halves of all (c, j) cover every step exactly once).
                # All-16-bit: i16 compare -> bf16 mask -> bf16 mult/reduce.
                iview = iota16[:].rearrange("b (l t) -> b l t", t=TAGSET)
                oh = ohp.tile([BC, CH * TAGSET], bf16, tag="oh")
                ov = oh[:].rearrange("b (l t) -> b l t", t=TAGSET)
                base = j * L + c * CH
                tsl = tags16[:, base:base + CH]
                tbc = tsl.rearrange("b l -> b l ()").to_broadcast(
                    [BC, CH, TAGSET])
                nc.vector.tensor_tensor(out=ov[:], in0=iview, in1=tbc,
                                        op=OP.is_equal)
                cslice = combx[:].rearrange(
                    "b (l j t) -> b l j t", j=NSEG, t=TAGSET)[:, :, j, :]
                scrap = ohp.tile([BC, CH * TAGSET], bf16, tag="scrap")
                col = c * NSEG + j
                nc.vector.tensor_mul(scrap[:], oh[:], cslice)
                nc.vector.tensor_reduce(
                    acc_e[:, col:col + 1], scrap[:], axis=AX.X, op=OP.add)

            # ---------------- prologue: chunk 0 xt ----------------
            comb_cur, combx_cur = load_chunk(0)
            xts = [prep_tick(comb_cur, l) for l in range(CH)]

            # ---------------- main scan: 256 ticks ----------------
            for tau in range(L):
                c, l = divmod(tau, CH)
                if l == 0 and c + 1 < NCHK:
                    comb_nxt, combx_nxt = load_chunk(c + 1)
                if c + 1 < NCHK:
                    xts.append(prep_tick(comb_nxt, l))
                if gold >= 2 and l % 2 == 1:
                    gold_piece(combx_cur, c, l // 2)

                ps = mmp.tile([64, NC], f32, tag="mm")
                nc.tensor.matmul(ps[:], wb16[:], state[:], start=True,
                                 stop=True)
                nstate = stp.tile([64, NC], bf16, tag="state")
                nc.vector.tensor_mul(nstate[:], ps[:], xts[tau])
                state = nstate

                if tau == L // 2 - 1:
                    # renorm: rescale every column by its proxy-row value;
                    # only fwd log-offsets matter (h offsets cancel in the
                    # stitch's dn/dd ratio)
                    pr = smp.tile([64, NC], f32, tag="pr")
                    nc.vector.tensor_copy(pr[0:1, :], state[0:1, :])
                    nc.vector.tensor_copy(pr[32:33, :], state[32:33, :])
                    rec = smp.tile([64, NC], f32, tag="rec")
                    nc.vector.reciprocal(rec[0:1, :], pr[0:1, :])
                    nc.vector.reciprocal(rec[32:33, :], pr[32:33, :])
                    bc_ps = fip.tile([64, NC], f32, tag="bc")
                    nc.tensor.matmul(bc_ps[0:32, :], ones_t[0:1, 0:32],
                                     rec[0:1, :], start=True, stop=True)
                    nc.tensor.matmul(bc_ps[32:64, :], ones_t[32:33, 0:32],
                                     rec[32:33, :], start=True, stop=True,
                                     tile_position=(32, 32))
                    lg = smp.tile([64, NC], f32, tag="lg")
                    nc.scalar.activation(lg[0:1, :], pr[0:1, :], AF.Ln)
                    nc.vector.tensor_add(offacc[0:1, :], offacc[0:1, :],
                                         lg[0:1, :])
                    rstate = stp.tile([64, NC], bf16, tag="state")
                    nc.vector.tensor_mul(rstate[:], state[:], bc_ps[:])
                    state = rstate
                if l == CH - 1:
                    comb_cur = comb_nxt
                    combx_cur = combx_nxt

            # ---------------- stitch ----------------
            # phi = M f (fwd-block matmul on the fwd finals)
            phi = fip.tile([32, NC], f32, tag="fin")
            nc.tensor.matmul(phi[:], wb16[0:32, 0:32], state[0:32, :],
                             start=True, stop=True)
            hs = smp.tile([32, NC], bf16, tag="hs")
            nc.sync.dma_start(hs[:], state[32:64, :])
            DN = (NSEG - 1) * 64
            pk = smp.tile([32, 2 * NC], f32, tag="pk")
            nc.vector.memset(pk[:], 1.0)
            # dn_k = h_k . (M f_{k-1}):  h groups 1..7 vs phi groups 0..6
            nc.vector.tensor_mul(pk[:, 0:DN], phi[:, 0:DN], hs[:, 64:NC])
            # dd_k = h_k . c
            nc.vector.tensor_mul(pk[:, NC:NC + DN], hs[:, 64:NC],
                                 c0[:, 0:1].to_broadcast([32, DN]))
            # d0 = w . f_m
            nc.vector.tensor_mul(pk[:, NC + DN:2 * NC],
                                 state[0:32, NC - 64:NC],
                                 wv[:, 0:1].to_broadcast([32, 64]))
            sumA = fip.tile([1, NC], f32, tag="fin")
            nc.tensor.matmul(sumA[:], ones_t[0:32, 0:1], pk[:, 0:NC],
                             start=True, stop=True)
            lnA = smp.tile([1, NC], f32, tag="lnA")
            nc.scalar.activation(lnA[:], sumA[:], AF.Ln)
            sumB = fip.tile([1, NC], f32, tag="fin")
            nc.tensor.matmul(sumB[:], ones_t[0:32, 0:1], pk[:, NC:2 * NC],
                             start=True, stop=True)
            lnB = smp.tile([1, NC], f32, tag="lnB")
            nc.scalar.activation(lnB[:], sumB[:], AF.Ln)

            lz = smp.tile([1, 64], f32, tag="lz")
            # logZ = d0ln + sum_x dn - sum_x dd + sum_k off_f + MU*S
            nc.vector.tensor_copy(lz[:], lnB[:, DN:NC])
            dnsum = smp.tile([1, 64], f32, tag="dnsum")
            nc.vector.tensor_reduce(
                dnsum[:], lnA[:, 0:DN].rearrange("p (x b) -> p b x", b=64),
                axis=AX.X, op=OP.add)
            ddsum = smp.tile([1, 64], f32, tag="ddsum")
            nc.vector.tensor_reduce(
                ddsum[:], lnB[:, 0:DN].rearrange("p (x b) -> p b x", b=64),
                axis=AX.X, op=OP.add)
            offsum = smp.tile([1, 64], f32, tag="offsum")
            nc.vector.tensor_reduce(
                offsum[:], offacc[0:1, :].rearrange("p (k b) -> p b k", b=64),
                axis=AX.X, op=OP.add)
            nc.vector.tensor_add(lz[:], lz[:], dnsum[:])
            nc.vector.tensor_sub(lz[:], lz[:], ddsum[:])
            nc.vector.tensor_add(lz[:], lz[:], offsum[:])
            # gold emission score
            gold_c = cp.tile([BC, 1], f32)
            nc.vector.tensor_reduce(gold_c[:], acc_e[:], axis=AX.X, op=OP.add)
            goldT = fip.tile([1, 64], f32, tag="fin")
            nc.tensor.transpose(goldT[0:1, :], gold_c[:, 0:1], ident[:])
            nc.vector.tensor_sub(lz[:], lz[:], goldT[0:1, :])
            nc.vector.tensor_scalar_add(lz[:], lz[:], float(MU) * S)
            nc.sync.dma_start(nll_d, lz[:])

            if _loop is not None:
                _loop.__exit__(None, None, None)

    nc.compile()
    return nc


def _get_nc():
    if "nc" not in _CACHE:
        _CACHE["nc"] = _build_nc()
    return _CACHE["nc"]


def kernel(emissions, transitions, tags):
    from concourse.bass_utils import run_bass_kernel_spmd

    em = np.ascontiguousarray(np.asarray(emissions, dtype=np.float32))
    tr = np.ascontiguousarray(np.asarray(transitions, dtype=np.float32))
    tg = np.ascontiguousarray(np.asarray(tags, dtype=np.int32))

    nc = _get_nc()
    in_maps = [
        {
            "emissions": em[c * BC:(c + 1) * BC],
            "tags": tg[c * BC:(c + 1) * BC],
            "transitions": tr,
        }
        for c in range(NCORES)
    ]
    res = run_bass_kernel_spmd(nc, in_maps, list(range(NCORES)))
    nll = np.concatenate([res.results[c]["nll"][0] for c in range(NCORES)])
    t_sc = (tr[tg[:, 1:], tg[:, :-1]].sum(axis=1)
            + tr[tg[:, 0], START] + tr[STOP, tg[:, -1]])
    total = np.sum(nll.astype(np.float64)) - np.sum(t_sc.astype(np.float64))
    return np.array(total, dtype=np.float32)


# revision 43
# speedup vs baseline: 1.4659x; 1.4659x over previous
"""CRF negative-log-likelihood loss (BERT_BiLSTM_CRF) on 8 TRN2 NeuronCores.

v4: rank-1 segment factorization (data-parallel over batch, 64 seqs/core).

The linear-space forward recursion a_t = D_t M a_{t-1} is split into
NSEG=8 segments of L=256 steps. Products of positive matrices contract
to rank-1 (Birkhoff), so each segment's transfer matrix P_k is summarized
by probe chains computed with the SAME per-tick kernel structure:
  f_k = P_k e   (forward chain; f_1 starts from the true one-hot START)
  h_k: u <- D_t M^T u over the segment, descending   ((M^T e)^T P_k = h_k^T M)
Stitch:  logZ = log(w^T f_m) + sum_{k>=2}[log(h_k^T M f_{k-1})
                                          - log(h_k^T M e)] + offsets + MU*S
(verified exact to 1e-11 in fp64 on the real data: the rank-2 component
of a 256-step product is ~0).

All 15 chains run CONCURRENTLY: state is [64, 512] bf16 = (fwd tags |
bwd tags) x (segment, sequence), so the serial scan is 256 ticks of one
bf16 matmul [64,512] + one DVE multiply — 4x fewer serial round trips
than meet-in-the-middle. Per tick, 8 PE transposes + 1 batched ACT exp
produce the tick's xt [64,512] from chunked emission DMAs (fwd natural,
bwd reversed), a chunk ahead of use. Gold emission score via one-hot
pieces on DVE spread between ticks; renorm once at tick 127 by proxy-row
reciprocal (h-chain scale offsets cancel in the stitch, so only f-chain
logs accumulate).
"""
import numpy as np

TAGSET = 32
START = 30
STOP = 31
B = 512
S = 2048
NCORES = 8
BC = B // NCORES          # 64 sequences per core
NSEG = 8                  # segments (rank-1 factorization)
L = S // NSEG             # 256 serial ticks
CH = 16                   # ticks per emission chunk
NCHK = L // CH            # 16 chunks
MU = np.float32(4.3226)   # mean log-growth per step (measured offline)

_CACHE = {}


def _build_nc(debug=False, gold=2, reps=1):
    import concourse.bacc as bacc
    import concourse.bass as bass
    import concourse.tile as tile
    from concourse import mybir

    f32 = mybir.dt.float32
    bf16 = mybir.dt.bfloat16
    i32 = mybir.dt.int32
    AF = mybir.ActivationFunctionType
    OP = mybir.AluOpType
    AX = mybir.AxisListType

    nc = bacc.Bacc("TRN2", target_bir_lowering=False, debug=False,
                   num_devices=NCORES)

    em_d = nc.dram_tensor("emissions", [BC, S, TAGSET], f32,
                          kind="ExternalInput").ap()
    tg_d = nc.dram_tensor("tags", [BC, S], i32, kind="ExternalInput").ap()
    tr_d = nc.dram_tensor("transitions", [TAGSET, TAGSET], f32,
                          kind="ExternalInput").ap()
    nll_d = nc.dram_tensor("nll", [1, BC], f32, kind="ExternalOutput").ap()

    NC = NSEG * 64            # state columns
    # emissions viewed [b, (seg, step), tag]
    em_r = em_d.rearrange("b (j s) t -> b s j t", j=NSEG)

    with tile.TileContext(nc) as tc:
        with (
            tc.tile_pool(name="const", bufs=1) as cp,
            tc.tile_pool(name="chunk", bufs=2) as ccp,
            tc.tile_pool(name="oh", bufs=2) as ohp,
            tc.tile_pool(name="xt", bufs=CH + 3) as xtp,
            tc.tile_pool(name="state", bufs=3) as stp,
            tc.tile_pool(name="small", bufs=2) as smp,
            tc.tile_pool(name="trp", bufs=3, space="PSUM") as trp,
            tc.tile_pool(name="mmp", bufs=2, space="PSUM") as mmp,
            tc.tile_pool(name="finp", bufs=1, space="PSUM") as fip,
        ):
            _loop = None
            if reps > 1:
                _loop = tc.For_i(
                    0, reps, 1,
                    hint_engines=(mybir.EngineType.PE, mybir.EngineType.DVE,
                                  mybir.EngineType.Activation))
                _loop.__enter__()

            # ---------------- setup: weights, identity, ones ----------------
            w = cp.tile([64, 64], f32)
            nc.vector.memset(w[:], 0.0)
            # fwd block: w[p, t] = trans[t, p] -> applies M = exp(trans)
            nc.sync.dma_start(w[0:32, 0:32], tr_d.rearrange("a b -> b a"))
            # bwd block: w[32+p, 32+t] = trans[p, t] -> applies M^T
            nc.sync.dma_start(w[32:64, 32:64], tr_d)
            nc.vector.tensor_scalar_max(w[:], w[:], -80.0)
            nc.scalar.activation(w[:], w[:], AF.Exp)
            nc.vector.memset(w[0:32, 32:64], 0.0)
            nc.vector.memset(w[32:64, 0:32], 0.0)
            wb16 = cp.tile([64, 64], bf16)
            nc.vector.tensor_copy(wb16[:], w[:])

            ones_t = cp.tile([64, 64], f32)
            nc.vector.memset(ones_t[:], 1.0)
            negmu = cp.tile([64, 1], f32)
            nc.vector.memset(negmu[:], -float(MU))
            ident = cp.tile([64, 64], f32)
            nc.gpsimd.affine_select(
                out=ident[:], in_=ones_t[:], pattern=[[-1, 64]],
                compare_op=OP.is_equal, fill=0.0, base=0, channel_multiplier=1)
            ident_b = cp.tile([64, 64], bf16)
            nc.vector.tensor_copy(ident_b[:], ident[:])

            # stitch constants: c = M e (bwd-block row sums), w_stop vector
            cvec = cp.tile([64, 1], f32)
            nc.vector.tensor_reduce(cvec[32:64, :], w[32:64, 32:64],
                                    axis=AX.X, op=OP.add)
            c0 = cp.tile([32, 1], f32)
            nc.sync.dma_start(c0[:], cvec[32:64, :])
            wv = cp.tile([32, 1], f32)
            nc.sync.dma_start(wv[:], tr_d[STOP:STOP + 1, :]
                              .rearrange("a b -> b a"))
            nc.vector.tensor_scalar_max(wv[:], wv[:], -80.0)
            nc.scalar.activation(wv[:], wv[:], AF.Exp)

            # ---------------- gold one-hot machinery ----------------
            # Gold runs two one-hot pieces per DVE op on 128 partitions
            # (rows 0-63: segment 2p for batch b; rows 64-127: the SAME
            # batch again, with tags/emissions copies PRE-SHIFTED by one
            # segment so one affine AP covers both halves).
            i16 = mybir.dt.int16
            tags_sb = cp.tile([2 * BC, S], i32)
            nc.sync.dma_start(tags_sb[0:BC, :], tg_d)
            nc.sync.dma_start(tags_sb[BC:2 * BC, 0:S - L], tg_d[:, L:S])
            nc.vector.memset(tags_sb[BC:2 * BC, S - L:S], 0.0)
            tags16 = cp.tile([2 * BC, S], i16)
            nc.vector.tensor_copy(tags16[:], tags_sb[:])
            iota_t = cp.tile([2 * BC, CH * TAGSET], i32)
            nc.gpsimd.iota(iota_t[:], pattern=[[0, CH], [1, TAGSET]], base=0,
                           channel_multiplier=0)
            iota16 = cp.tile([2 * BC, CH * TAGSET], i16)
            nc.vector.tensor_copy(iota16[:], iota_t[:])
            NACC = NCHK * (NSEG // 2)
            acc_e = cp.tile([2 * BC, NACC], f32)
            nc.vector.memset(acc_e[:], 0.0)

            # ---------------- state init ----------------
            offacc = cp.tile([64, NC], f32)
            nc.vector.memset(offacc[:], 0.0)
            ones_b = cp.tile([64, 64], bf16)
            nc.vector.memset(ones_b[:], 1.0)

            state = stp.tile([64, NC], bf16, tag="state")
            nc.vector.memset(state[:], 1.0)
            # f_1 init: one-hot START on fwd group 0
            nc.gpsimd.affine_select(
                out=state[0:32, 0:64], in_=ones_b[0:32, :], pattern=[[0, 64]],
                compare_op=OP.is_equal, fill=0.0, base=-START,
                channel_multiplier=1)

            # ---------------- chunk machinery ----------------
            # comb(c): [BC, CH*NSEG*64] f32, dims (l, j, u, t):
            #   u=0: emissions[:, j*L + c*CH + l, :]          (fwd, ascending)
            #   u=1: emissions[:, (j+1)*L - 1 - c*CH - l, :]  (bwd, descending)
            def load_chunk(c):
                comb = ccp.tile([BC, CH * NSEG * 64], f32, tag="comb")
                cv = comb[:].rearrange("b (l j u t) -> b l j u t",
                                       j=NSEG, u=2, t=TAGSET)
                hi = L - 1 - c * CH
                lo = L - (c + 1) * CH - 1
                bsl = slice(hi, None, -1) if lo < 0 else slice(hi, lo, -1)
                for j in range(NSEG):
                    nc.sync.dma_start(cv[:, :, j, 0, :],
                                      em_r[:, c * CH:(c + 1) * CH, j, :])
                    nc.sync.dma_start(cv[:, :, j, 1, :], em_r[:, bsl, j, :])
                # bf16 copy of the fwd halves for the 16-bit gold pipeline;
                # upper partition half holds the NEXT segment's data (SBUF
                # partition-shift DMAs) so gold pairs two segments per op.
                combx = ccp.tile([2 * BC, CH * NSEG * TAGSET], bf16,
                                 tag="combx")
                xv = combx[:].rearrange("b (l j t) -> b l j t",
                                        j=NSEG, t=TAGSET)
                nc.scalar.activation(xv[0:BC, :, 0:4, :],
                                     cv[:, :, 0:4, 0, :], AF.Copy)
                nc.scalar.activation(xv[0:BC, :, 4:8, :],
                                     cv[:, :, 4:8, 0, :], AF.Copy)
                for p in range(NSEG // 2):
                    nc.sync.dma_start(xv[BC:2 * BC, :, 2 * p, :],
                                      xv[0:BC, :, 2 * p + 1, :])
                return comb, combx

            # per-tick xt: 8 transposes [64,64] -> one PSUM bank -> one exp
            def prep_tick(comb, l):
                trg = trp.tile([64, NC], f32, tag="trg")
                for j in range(NSEG):
                    nc.tensor.transpose(
                        trg[:, j * 64:(j + 1) * 64],
                        comb[:, (l * NSEG + j) * 64:(l * NSEG + j + 1) * 64],
                        ident[:])
                xt = xtp.tile([64, NC], bf16, tag="xt")
                nc.scalar.activation(xt[:], trg[:], AF.Exp, bias=negmu[:])
                return xt

            def gold_piece(combx, c, p):
                # one-hot gold for chunk c, segments (2p, 2p+1) in ONE
                # 128-partition op pair: rows 0-63 do segment 2p, rows
                # 64-127 do segment 2p+1 via the pre-shifted copies.
                # All-16-bit: i16 compare -> bf16 mask -> bf16 mult/reduce.
                iview = iota16[:].rearrange("b (l t) -> b l t", t=TAGSET)
                oh = ohp.tile([2 * BC, CH * TAGSET], bf16, tag="oh")
                ov = oh[:].rearrange("b (l t) -> b l t", t=TAGSET)
                base = 2 * p * L + c * CH
                tsl = tags16[:, base:base + CH]
                tbc = tsl.rearrange("b l -> b l ()").to_broadcast(
                    [2 * BC, CH, TAGSET])
                nc.vector.tensor_tensor(out=ov[:], in0=iview, in1=tbc,
                                        op=OP.is_equal)
                cslice = combx[:].rearrange(
                    "b (l j t) -> b l j t", j=NSEG, t=TAGSET)[:, :, 2 * p, :]
                scrap = ohp.tile([2 * BC, CH * TAGSET], bf16, tag="scrap")
                col = c * (NSEG // 2) + p
                nc.vector.tensor_mul(scrap[:], oh[:], cslice)
                nc.vector.tensor_reduce(
                    acc_e[:, col:col + 1], scrap[:], axis=AX.X, op=OP.add)

            # ---------------- prologue: chunk 0 xt ----------------
            comb_cur, combx_cur = load_chunk(0)
            xts = [prep_tick(comb_cur, l) for l in range(CH)]

            # ---------------- main scan: 256 ticks ----------------
            for tau in range(L):
                c, l = divmod(tau, CH)
                if l == 0 and c + 1 < NCHK:
                    comb_nxt, combx_nxt = load_chunk(c + 1)
                if c + 1 < NCHK:
                    xts.append(prep_tick(comb_nxt, l))
                if gold >= 2 and l % 4 == 1:
                    gold_piece(combx_cur, c, l // 4)

                ps = mmp.tile([64, NC], f32, tag="mm")
                nc.tensor.matmul(ps[:], wb16[:], state[:], start=True,
                                 stop=True)
                nstate = stp.tile([64, NC], bf16, tag="state")
                nc.vector.tensor_mul(nstate[:], ps[:], xts[tau])
                state = nstate

                if tau == L // 2 - 1:
                    # renorm: rescale every column by its proxy-row value;
                    # only fwd log-offsets matter (h offsets cancel in the
                    # stitch's dn/dd ratio)
                    pr = smp.tile([64, NC], f32, tag="pr")
                    nc.vector.tensor_copy(pr[0:1, :], state[0:1, :])
                    nc.vector.tensor_copy(pr[32:33, :], state[32:33, :])
                    rec = smp.tile([64, NC], f32, tag="rec")
                    nc.vector.reciprocal(rec[0:1, :], pr[0:1, :])
                    nc.vector.reciprocal(rec[32:33, :], pr[32:33, :])
                    bc_ps = fip.tile([64, NC], f32, tag="bc")
                    nc.tensor.matmul(bc_ps[0:32, :], ones_t[0:1, 0:32],
                                     rec[0:1, :], start=True, stop=True)
                    nc.tensor.matmul(bc_ps[32:64, :], ones_t[32:33, 0:32],
                                     rec[32:33, :], start=True, stop=True,
                                     tile_position=(32, 32))
                    lg = smp.tile([64, NC], f32, tag="lg")
                    nc.scalar.activation(lg[0:1, :], pr[0:1, :], AF.Ln)
                    nc.vector.tensor_add(offacc[0:1, :], offacc[0:1, :],
                                         lg[0:1, :])
                    rstate = stp.tile([64, NC], bf16, tag="state")
                    nc.vector.tensor_mul(rstate[:], state[:], bc_ps[:])
                    state = rstate
                if l == CH - 1:
                    comb_cur = comb_nxt
                    combx_cur = combx_nxt

            # ---------------- stitch ----------------
            # phi = M f (fwd-block matmul on the fwd finals)
            phi = fip.tile([32, NC], f32, tag="fin")
            nc.tensor.matmul(phi[:], wb16[0:32, 0:32], state[0:32, :],
                             start=True, stop=True)
            hs = smp.tile([32, NC], bf16, tag="hs")
            nc.sync.dma_start(hs[:], state[32:64, :])
            DN = (NSEG - 1) * 64
            pk = smp.tile([32, 2 * NC], f32, tag="pk")
            nc.vector.memset(pk[:], 1.0)
            # dn_k = h_k . (M f_{k-1}):  h groups 1..7 vs phi groups 0..6
            nc.vector.tensor_mul(pk[:, 0:DN], phi[:, 0:DN], hs[:, 64:NC])
            # dd_k = h_k . c
            nc.vector.tensor_mul(pk[:, NC:NC + DN], hs[:, 64:NC],
                                 c0[:, 0:1].to_broadcast([32, DN]))
            # d0 = w . f_m
            nc.vector.tensor_mul(pk[:, NC + DN:2 * NC],
                                 state[0:32, NC - 64:NC],
                                 wv[:, 0:1].to_broadcast([32, 64]))
            sumA = fip.tile([1, NC], f32, tag="fin")
            nc.tensor.matmul(sumA[:], ones_t[0:32, 0:1], pk[:, 0:NC],
                             start=True, stop=True)
            lnA = smp.tile([1, NC], f32, tag="lnA")
            nc.scalar.activation(lnA[:], sumA[:], AF.Ln)
            sumB = fip.tile([1, NC], f32, tag="fin")
            nc.tensor.matmul(sumB[:], ones_t[0:32, 0:1], pk[:, NC:2 * NC],
                             start=True, stop=True)
            lnB = smp.tile([1, NC], f32, tag="lnB")
            nc.scalar.activation(lnB[:], sumB[:], AF.Ln)

            lz = smp.tile([1, 64], f32, tag="lz")
            # logZ = d0ln + sum_x dn - sum_x dd + sum_k off_f + MU*S
            nc.vector.tensor_copy(lz[:], lnB[:, DN:NC])
            dnsum = smp.tile([1, 64], f32, tag="dnsum")
            nc.vector.tensor_reduce(
                dnsum[:], lnA[:, 0:DN].rearrange("p (x b) -> p b x", b=64),
                axis=AX.X, op=OP.add)
            ddsum = smp.tile([1, 64], f32, tag="ddsum")
            nc.vector.tensor_reduce(
                ddsum[:], lnB[:, 0:DN].rearrange("p (x b) -> p b x", b=64),
                axis=AX.X, op=OP.add)
            offsum = smp.tile([1, 64], f32, tag="offsum")
            nc.vector.tensor_reduce(
                offsum[:], offacc[0:1, :].rearrange("p (k b) -> p b k", b=64),
                axis=AX.X, op=OP.add)
            nc.vector.tensor_add(lz[:], lz[:], dnsum[:])
            nc.vector.tensor_sub(lz[:], lz[:], ddsum[:])
            nc.vector.tensor_add(lz[:], lz[:], offsum[:])
            # gold emission score: fold the upper-half accumulators in
            g2 = cp.tile([2 * BC, 1], f32)
            nc.vector.tensor_reduce(g2[:], acc_e[:], axis=AX.X, op=OP.add)
            gu = cp.tile([BC, 1], f32)
            nc.sync.dma_start(gu[:], g2[BC:2 * BC, :])
            gold_c = cp.tile([BC, 1], f32)
            nc.vector.tensor_add(gold_c[:], g2[0:BC, :], gu[:])
            goldT = fip.tile([1, 64], f32, tag="fin")
            nc.tensor.transpose(goldT[0:1, :], gold_c[:, 0:1], ident[:])
            nc.vector.tensor_sub(lz[:], lz[:], goldT[0:1, :])
            nc.vector.tensor_scalar_add(lz[:], lz[:], float(MU) * S)
            nc.sync.dma_start(nll_d, lz[:])

            if _loop is not None:
                _loop.__exit__(None, None, None)

    nc.compile()
    return nc


def _get_nc():
    if "nc" not in _CACHE:
        _CACHE["nc"] = _build_nc()
    return _CACHE["nc"]


def kernel(emissions, transitions, tags):
    from concourse.bass_utils import run_bass_kernel_spmd

    em = np.ascontiguousarray(np.asarray(emissions, dtype=np.float32))
    tr = np.ascontiguousarray(np.asarray(transitions, dtype=np.float32))
    tg = np.ascontiguousarray(np.asarray(tags, dtype=np.int32))

    nc = _get_nc()
    in_maps = [
        {
            "emissions": em[c * BC:(c + 1) * BC],
            "tags": tg[c * BC:(c + 1) * BC],
            "transitions": tr,
        }
        for c in range(NCORES)
    ]
    res = run_bass_kernel_spmd(nc, in_maps, list(range(NCORES)))
    nll = np.concatenate([res.results[c]["nll"][0] for c in range(NCORES)])
    t_sc = (tr[tg[:, 1:], tg[:, :-1]].sum(axis=1)
            + tr[tg[:, 0], START] + tr[STOP, tg[:, -1]])
    total = np.sum(nll.astype(np.float64)) - np.sum(t_sc.astype(np.float64))
    return np.array(total, dtype=np.float32)


# revision 48
# speedup vs baseline: 1.5026x; 1.0250x over previous
"""CRF negative-log-likelihood loss (BERT_BiLSTM_CRF) on 8 TRN2 NeuronCores.

v4: rank-1 segment factorization (data-parallel over batch, 64 seqs/core).

The linear-space forward recursion a_t = D_t M a_{t-1} is split into
NSEG=8 segments of L=256 steps. Products of positive matrices contract
to rank-1 (Birkhoff), so each segment's transfer matrix P_k is summarized
by probe chains computed with the SAME per-tick kernel structure:
  f_k = P_k e   (forward chain; f_1 starts from the true one-hot START)
  h_k: u <- D_t M^T u over the segment, descending   ((M^T e)^T P_k = h_k^T M)
Stitch:  logZ = log(w^T f_m) + sum_{k>=2}[log(h_k^T M f_{k-1})
                                          - log(h_k^T M e)] + offsets + MU*S
(verified exact to 1e-11 in fp64 on the real data: the rank-2 component
of a 256-step product is ~0).

All 15 chains run CONCURRENTLY: state is [64, 512] bf16 = (fwd tags |
bwd tags) x (segment, sequence), so the serial scan is 256 ticks of one
bf16 matmul [64,512] + one DVE multiply — 4x fewer serial round trips
than meet-in-the-middle. Per tick, 8 PE transposes + 1 batched ACT exp
produce the tick's xt [64,512] from chunked emission DMAs (fwd natural,
bwd reversed), a chunk ahead of use. Gold emission score via one-hot
pieces on DVE spread between ticks; renorm once at tick 127 by proxy-row
reciprocal (h-chain scale offsets cancel in the stitch, so only f-chain
logs accumulate).
"""
import numpy as np

TAGSET = 32
START = 30
STOP = 31
B = 512
S = 2048
NCORES = 8
BC = B // NCORES          # 64 sequences per core
NSEG = 8                  # segments (rank-1 factorization)
L = S // NSEG             # 256 serial ticks
CH = 16                   # ticks per emission chunk
NCHK = L // CH            # 16 chunks
MU = np.float32(4.3226)   # mean log-growth per step (measured offline)

_CACHE = {}


def _build_nc(debug=False, gold=2, reps=1):
    import concourse.bacc as bacc
    import concourse.bass as bass
    import concourse.tile as tile
    from concourse import mybir

    f32 = mybir.dt.float32
    bf16 = mybir.dt.bfloat16
    i32 = mybir.dt.int32
    AF = mybir.ActivationFunctionType
    OP = mybir.AluOpType
    AX = mybir.AxisListType

    nc = bacc.Bacc("TRN2", target_bir_lowering=False, debug=False,
                   num_devices=NCORES)

    em_d = nc.dram_tensor("emissions", [BC, S, TAGSET], f32,
                          kind="ExternalInput").ap()
    tg_d = nc.dram_tensor("tags", [BC, S], i32, kind="ExternalInput").ap()
    tr_d = nc.dram_tensor("transitions", [TAGSET, TAGSET], f32,
                          kind="ExternalInput").ap()
    nll_d = nc.dram_tensor("nll", [1, BC], f32, kind="ExternalOutput").ap()

    NC = NSEG * 64            # state columns
    # emissions viewed [b, (seg, step), tag]
    em_r = em_d.rearrange("b (j s) t -> b s j t", j=NSEG)

    with tile.TileContext(nc) as tc:
        with (
            tc.tile_pool(name="const", bufs=1) as cp,
            tc.tile_pool(name="chunk", bufs=2) as ccp,
            tc.tile_pool(name="oh", bufs=2) as ohp,
            tc.tile_pool(name="xt", bufs=CH + 3) as xtp,
            tc.tile_pool(name="state", bufs=3) as stp,
            tc.tile_pool(name="small", bufs=2) as smp,
            tc.tile_pool(name="trp", bufs=2, space="PSUM") as trp,
            tc.tile_pool(name="mmp", bufs=2, space="PSUM") as mmp,
            tc.tile_pool(name="finp", bufs=1, space="PSUM") as fip,
        ):
            _loop = None
            if reps > 1:
                _loop = tc.For_i(
                    0, reps, 1,
                    hint_engines=(mybir.EngineType.PE, mybir.EngineType.DVE,
                                  mybir.EngineType.Activation))
                _loop.__enter__()

            # ---------------- setup: weights, identity, ones ----------------
            w = cp.tile([64, 64], f32)
            nc.vector.memset(w[:], 0.0)
            # fwd block: w[p, t] = trans[t, p] -> applies M = exp(trans)
            nc.sync.dma_start(w[0:32, 0:32], tr_d.rearrange("a b -> b a"))
            # bwd block: w[32+p, 32+t] = trans[p, t] -> applies M^T
            nc.sync.dma_start(w[32:64, 32:64], tr_d)
            nc.vector.tensor_scalar_max(w[:], w[:], -80.0)
            nc.scalar.activation(w[:], w[:], AF.Exp)
            nc.vector.memset(w[0:32, 32:64], 0.0)
            nc.vector.memset(w[32:64, 0:32], 0.0)
            wb16 = cp.tile([64, 64], bf16)
            nc.vector.tensor_copy(wb16[:], w[:])

            ones_t = cp.tile([64, 64], f32)
            nc.vector.memset(ones_t[:], 1.0)
            negmu = cp.tile([64, 1], f32)
            nc.vector.memset(negmu[:], -float(MU))
            ident = cp.tile([64, 64], f32)
            nc.gpsimd.affine_select(
                out=ident[:], in_=ones_t[:], pattern=[[-1, 64]],
                compare_op=OP.is_equal, fill=0.0, base=0, channel_multiplier=1)
            ident_b = cp.tile([64, 64], bf16)
            nc.vector.tensor_copy(ident_b[:], ident[:])

            # stitch constants: c = M e (bwd-block row sums), w_stop vector
            cvec = cp.tile([64, 1], f32)
            nc.vector.tensor_reduce(cvec[32:64, :], w[32:64, 32:64],
                                    axis=AX.X, op=OP.add)
            c0 = cp.tile([32, 1], f32)
            nc.sync.dma_start(c0[:], cvec[32:64, :])
            wv = cp.tile([32, 1], f32)
            nc.sync.dma_start(wv[:], tr_d[STOP:STOP + 1, :]
                              .rearrange("a b -> b a"))
            nc.vector.tensor_scalar_max(wv[:], wv[:], -80.0)
            nc.scalar.activation(wv[:], wv[:], AF.Exp)

            # ---------------- gold one-hot machinery ----------------
            # Gold runs two one-hot pieces per DVE op on 128 partitions
            # (rows 0-63: segment 2p for batch b; rows 64-127: the SAME
            # batch again, with tags/emissions copies PRE-SHIFTED by one
            # segment so one affine AP covers both halves).
            i16 = mybir.dt.int16
            tags_sb = cp.tile([2 * BC, S], i32)
            nc.sync.dma_start(tags_sb[0:BC, :], tg_d)
            nc.sync.dma_start(tags_sb[BC:2 * BC, 0:S - L], tg_d[:, L:S])
            nc.vector.memset(tags_sb[BC:2 * BC, S - L:S], 0.0)
            tags16 = cp.tile([2 * BC, S], i16)
            nc.vector.tensor_copy(tags16[:], tags_sb[:])
            iota_t = cp.tile([2 * BC, CH * TAGSET], i32)
            nc.gpsimd.iota(iota_t[:], pattern=[[0, CH], [1, TAGSET]], base=0,
                           channel_multiplier=0)
            iota16 = cp.tile([2 * BC, CH * TAGSET], i16)
            nc.vector.tensor_copy(iota16[:], iota_t[:])
            NACC = NCHK * (NSEG // 2)
            acc_e = cp.tile([2 * BC, NACC], f32)
            nc.vector.memset(acc_e[:], 0.0)

            # ---------------- state init ----------------
            offacc = cp.tile([64, NC], f32)
            nc.vector.memset(offacc[:], 0.0)
            ones_b = cp.tile([64, 64], bf16)
            nc.vector.memset(ones_b[:], 1.0)

            # two half-chains (anti-phase): A = segment groups 0-3,
            # B = groups 4-7; separate tiles so each half's matmul only
            # depends on its own multiply (halves the serial round trip)
            HC = NC // 2
            stateA = stp.tile([64, HC], bf16, tag="stateA")
            nc.vector.memset(stateA[:], 1.0)
            stateB = stp.tile([64, HC], bf16, tag="stateB")
            nc.vector.memset(stateB[:], 1.0)
            # f_1 init: one-hot START on fwd group 0
            nc.gpsimd.affine_select(
                out=stateA[0:32, 0:64], in_=ones_b[0:32, :],
                pattern=[[0, 64]], compare_op=OP.is_equal, fill=0.0,
                base=-START, channel_multiplier=1)

            # ---------------- chunk machinery ----------------
            # comb(c): [BC, CH*NSEG*64] f32, dims (l, j, u, t):
            #   u=0: emissions[:, j*L + c*CH + l, :]          (fwd, ascending)
            #   u=1: emissions[:, (j+1)*L - 1 - c*CH - l, :]  (bwd, descending)
            def load_chunk(c):
                comb = ccp.tile([BC, CH * NSEG * 64], f32, tag="comb")
                cv = comb[:].rearrange("b (l j u t) -> b l j u t",
                                       j=NSEG, u=2, t=TAGSET)
                hi = L - 1 - c * CH
                lo = L - (c + 1) * CH - 1
                bsl = slice(hi, None, -1) if lo < 0 else slice(hi, lo, -1)
                for j in range(NSEG):
                    nc.sync.dma_start(cv[:, :, j, 0, :],
                                      em_r[:, c * CH:(c + 1) * CH, j, :])
                    nc.sync.dma_start(cv[:, :, j, 1, :], em_r[:, bsl, j, :])
                # bf16 copy of the fwd halves for the 16-bit gold pipeline;
                # upper partition half holds the NEXT segment's data (SBUF
                # partition-shift DMAs) so gold pairs two segments per op.
                combx = ccp.tile([2 * BC, CH * NSEG * TAGSET], bf16,
                                 tag="combx")
                xv = combx[:].rearrange("b (l j t) -> b l j t",
                                        j=NSEG, t=TAGSET)
                nc.scalar.activation(xv[0:BC, :, 0:4, :],
                                     cv[:, :, 0:4, 0, :], AF.Copy)
                nc.scalar.activation(xv[0:BC, :, 4:8, :],
                                     cv[:, :, 4:8, 0, :], AF.Copy)
                for p in range(NSEG // 2):
                    nc.sync.dma_start(xv[BC:2 * BC, :, 2 * p, :],
                                      xv[0:BC, :, 2 * p + 1, :])
                return comb, combx

            # per-tick xt: 8 transposes [64,64] -> one PSUM bank -> one exp
            def prep_tick(comb, l):
                trg = trp.tile([64, NC], f32, tag="trg")
                for j in range(NSEG):
                    nc.tensor.transpose(
                        trg[:, j * 64:(j + 1) * 64],
                        comb[:, (l * NSEG + j) * 64:(l * NSEG + j + 1) * 64],
                        ident[:])
                xt = xtp.tile([64, NC], bf16, tag="xt")
                nc.scalar.activation(xt[:], trg[:], AF.Exp, bias=negmu[:])
                return xt

            def gold_piece(combx, c, p):
                # one-hot gold for chunk c, segments (2p, 2p+1) in ONE
                # 128-partition op pair: rows 0-63 do segment 2p, rows
                # 64-127 do segment 2p+1 via the pre-shifted copies.
                # All-16-bit: i16 compare -> bf16 mask -> bf16 mult/reduce.
                iview = iota16[:].rearrange("b (l t) -> b l t", t=TAGSET)
                oh = ohp.tile([2 * BC, CH * TAGSET], bf16, tag="oh")
                ov = oh[:].rearrange("b (l t) -> b l t", t=TAGSET)
                base = 2 * p * L + c * CH
                tsl = tags16[:, base:base + CH]
                tbc = tsl.rearrange("b l -> b l ()").to_broadcast(
                    [2 * BC, CH, TAGSET])
                nc.vector.tensor_tensor(out=ov[:], in0=iview, in1=tbc,
                                        op=OP.is_equal)
                cslice = combx[:].rearrange(
                    "b (l j t) -> b l j t", j=NSEG, t=TAGSET)[:, :, 2 * p, :]
                scrap = ohp.tile([2 * BC, CH * TAGSET], bf16, tag="scrap")
                col = c * (NSEG // 2) + p
                nc.vector.tensor_mul(scrap[:], oh[:], cslice)
                nc.vector.tensor_reduce(
                    acc_e[:, col:col + 1], scrap[:], axis=AX.X, op=OP.add)

            # ---------------- prologue: chunk 0 xt ----------------
            comb_cur, combx_cur = load_chunk(0)
            xts = [prep_tick(comb_cur, l) for l in range(CH)]

            # ---------------- main scan: 256 ticks ----------------
            for tau in range(L):
                c, l = divmod(tau, CH)
                if l == 0 and c + 1 < NCHK:
                    comb_nxt, combx_nxt = load_chunk(c + 1)
                if c + 1 < NCHK:
                    xts.append(prep_tick(comb_nxt, l))
                if gold >= 2 and l % 4 == 1:
                    gold_piece(combx_cur, c, l // 4)

                psA = mmp.tile([64, HC], f32, tag="mmA")
                nc.tensor.matmul(psA[:], wb16[:], stateA[:], start=True,
                                 stop=True)
                nstateA = stp.tile([64, HC], bf16, tag="stateA")
                nc.vector.tensor_mul(nstateA[:], psA[:], xts[tau][:, 0:HC])
                stateA = nstateA
                psB = mmp.tile([64, HC], f32, tag="mmB")
                nc.tensor.matmul(psB[:], wb16[:], stateB[:], start=True,
                                 stop=True)
                nstateB = stp.tile([64, HC], bf16, tag="stateB")
                nc.vector.tensor_mul(nstateB[:], psB[:],
                                     xts[tau][:, HC:NC])
                stateB = nstateB

                if tau == L // 2 - 1:
                    # renorm: rescale every column by its proxy-row value;
                    # only fwd log-offsets matter (h offsets cancel in the
                    # stitch's dn/dd ratio)
                    for half, st in ((0, stateA), (1, stateB)):
                        pr = smp.tile([64, HC], f32, tag="pr")
                        nc.vector.tensor_copy(pr[0:1, :], st[0:1, :])
                        nc.vector.tensor_copy(pr[32:33, :], st[32:33, :])
                        rec = smp.tile([64, HC], f32, tag="rec")
                        nc.vector.reciprocal(rec[0:1, :], pr[0:1, :])
                        nc.vector.reciprocal(rec[32:33, :], pr[32:33, :])
                        bc_ps = fip.tile([64, HC], f32, tag="bc")
                        nc.tensor.matmul(bc_ps[0:32, :], ones_t[0:1, 0:32],
                                         rec[0:1, :], start=True, stop=True)
                        nc.tensor.matmul(bc_ps[32:64, :],
                                         ones_t[32:33, 0:32], rec[32:33, :],
                                         start=True, stop=True,
                                         tile_position=(32, 32))
                        lg = smp.tile([64, HC], f32, tag="lg")
                        nc.scalar.activation(lg[0:1, :], pr[0:1, :], AF.Ln)
                        nc.vector.tensor_add(
                            offacc[0:1, half * HC:(half + 1) * HC],
                            offacc[0:1, half * HC:(half + 1) * HC],
                            lg[0:1, :])
                        rst = stp.tile([64, HC], bf16,
                                       tag=("stateA", "stateB")[half])
                        nc.vector.tensor_mul(rst[:], st[:], bc_ps[:])
                        if half == 0:
                            stateA = rst
                        else:
                            stateB = rst
                if l == CH - 1:
                    comb_cur = comb_nxt
                    combx_cur = combx_nxt

            # ---------------- stitch ----------------
            # phi = M f (fwd-block matmul on the fwd finals)
            phi = fip.tile([32, NC], f32, tag="fin")
            nc.tensor.matmul(phi[:, 0:HC], wb16[0:32, 0:32], stateA[0:32, :],
                             start=True, stop=True)
            nc.tensor.matmul(phi[:, HC:NC], wb16[0:32, 0:32],
                             stateB[0:32, :], start=True, stop=True)
            hs = smp.tile([32, NC], bf16, tag="hs")
            nc.sync.dma_start(hs[:, 0:HC], stateA[32:64, :])
            nc.sync.dma_start(hs[:, HC:NC], stateB[32:64, :])
            DN = (NSEG - 1) * 64
            pk = smp.tile([32, 2 * NC], f32, tag="pk")
            nc.vector.memset(pk[:], 1.0)
            # dn_k = h_k . (M f_{k-1}):  h groups 1..7 vs phi groups 0..6
            nc.vector.tensor_mul(pk[:, 0:DN], phi[:, 0:DN], hs[:, 64:NC])
            # dd_k = h_k . c
            nc.vector.tensor_mul(pk[:, NC:NC + DN], hs[:, 64:NC],
                                 c0[:, 0:1].to_broadcast([32, DN]))
            # d0 = w . f_m  (group 7 = last group of the B half)
            nc.vector.tensor_mul(pk[:, NC + DN:2 * NC],
                                 stateB[0:32, HC - 64:HC],
                                 wv[:, 0:1].to_broadcast([32, 64]))
            sumA = fip.tile([1, NC], f32, tag="fin")
            nc.tensor.matmul(sumA[:], ones_t[0:32, 0:1], pk[:, 0:NC],
                             start=True, stop=True)
            lnA = smp.tile([1, NC], f32, tag="lnA")
            nc.scalar.activation(lnA[:], sumA[:], AF.Ln)
            sumB = fip.tile([1, NC], f32, tag="fin")
            nc.tensor.matmul(sumB[:], ones_t[0:32, 0:1], pk[:, NC:2 * NC],
                             start=True, stop=True)
            lnB = smp.tile([1, NC], f32, tag="lnB")
            nc.scalar.activation(lnB[:], sumB[:], AF.Ln)

            lz = smp.tile([1, 64], f32, tag="lz")
            # logZ = d0ln + sum_x dn - sum_x dd + sum_k off_f + MU*S
            nc.vector.tensor_copy(lz[:], lnB[:, DN:NC])
            dnsum = smp.tile([1, 64], f32, tag="dnsum")
            nc.vector.tensor_reduce(
                dnsum[:], lnA[:, 0:DN].rearrange("p (x b) -> p b x", b=64),
                axis=AX.X, op=OP.add)
            ddsum = smp.tile([1, 64], f32, tag="ddsum")
            nc.vector.tensor_reduce(
                ddsum[:], lnB[:, 0:DN].rearrange("p (x b) -> p b x", b=64),
                axis=AX.X, op=OP.add)
            offsum = smp.tile([1, 64], f32, tag="offsum")
            nc.vector.tensor_reduce(
                offsum[:], offacc[0:1, :].rearrange("p (k b) -> p b k", b=64),
                axis=AX.X, op=OP.add)
            nc.vector.tensor_add(lz[:], lz[:], dnsum[:])
            nc.vector.tensor_sub(lz[:], lz[:], ddsum[:])
            nc.vector.tensor_add(lz[:], lz[:], offsum[:])
            # gold emission score: fold the upper-half accumulators in
            g2 = cp.tile([2 * BC, 1], f32)
            nc.vector.tensor_reduce(g2[:], acc_e[:], axis=AX.X, op=OP.add)
            gu = cp.tile([BC, 1], f32)
            nc.sync.dma_start(gu[:], g2[BC:2 * BC, :])
            gold_c = cp.tile([BC, 1], f32)
            nc.vector.tensor_add(gold_c[:], g2[0:BC, :], gu[:])
            goldT = fip.tile([1, 64], f32, tag="fin")
            nc.tensor.transpose(goldT[0:1, :], gold_c[:, 0:1], ident[:])
            nc.vector.tensor_sub(lz[:], lz[:], goldT[0:1, :])
            nc.vector.tensor_scalar_add(lz[:], lz[:], float(MU) * S)
            nc.sync.dma_start(nll_d, lz[:])

            if _loop is not None:
                _loop.__exit__(None, None, None)

    nc.compile()
    return nc


def _get_nc():
    if "nc" not in _CACHE:
        _CACHE["nc"] = _build_nc()
    return _CACHE["nc"]


def kernel(emissions, transitions, tags):
    from concourse.bass_utils import run_bass_kernel_spmd

    em = np.ascontiguousarray(np.asarray(emissions, dtype=np.float32))
    tr = np.ascontiguousarray(np.asarray(transitions, dtype=np.float32))
    tg = np.ascontiguousarray(np.asarray(tags, dtype=np.int32))

    nc = _get_nc()
    in_maps = [
        {
            "emissions": em[c * BC:(c + 1) * BC],
            "tags": tg[c * BC:(c + 1) * BC],
            "transitions": tr,
        }
        for c in range(NCORES)
    ]
    res = run_bass_kernel_spmd(nc, in_maps, list(range(NCORES)))
    nll = np.concatenate([res.results[c]["nll"][0] for c in range(NCORES)])
    t_sc = (tr[tg[:, 1:], tg[:, :-1]].sum(axis=1)
            + tr[tg[:, 0], START] + tr[STOP, tg[:, -1]])
    total = np.sum(nll.astype(np.float64)) - np.sum(t_sc.astype(np.float64))
    return np.array(total, dtype=np.float32)
